# revision 66
# baseline (speedup 1.0000x reference)
"""Trainium2 Bass kernel for nn_ComposedCliffordSteerableKernel.

Computation (see reference): for each of 16x16 (m, n) block pairs, a tiny
3D conv (8,8,7^3) x (8,8,7^3) -> (8,8,7^3) with SAME padding, then
elementwise * shell * factor:

  out[m8+p, n8+q, od,oh,ow] =
      sum_{j,kd,kh,kw} k2[m8+q, n8+j, kd,kh,kw]
                     * k1[m8+p, n8+j, od+kd-3, oh+kh-3, ow+kw-3]

The PE is charged `output_free_size * cycles_per_row` per matmul
regardless of how many PE rows/columns are used, so the winning layout
maximizes contraction+output partitions per instruction and minimizes
streamed rows.  This kernel uses a *Toeplitz-in-depth* packing:

- PSUM partitions   = (nb, q, od)  : pair-in-duo, out blade, out depth = 112
- contraction rows  = (nb, j, id)  : pair-in-duo, in blade, abs. in depth = 112
- chunk loop        = (kh, kw)     : 49 accumulating matmuls per (m, duo)
- streamed free dim = (p, oh, ow)  : oh and ow restricted to the valid
                      window per (kh, kw) (win sums 37/49 per dim)

The kd contraction is absorbed into a host-precomputed block-diagonal
Toeplitz weight tile w[(nb,j,id),(nb,q,od)] = k2[q,j,id-od+3,kh,kw]
(zero off the n-diagonal and off the |id-od|<=3 band).  rhs is plain k1
with (nb,j,id) on partitions and (p,ih,iw) in-partition; the oh/ow
windows keep ih/iw interior, so no halo padding is needed.  Chunk
(kh=3,kw=3) runs first: its windows are full, so the accumulation
group's start=True matmul covers the whole PSUM tile.

fp16 operands (measured 7.3e-4 rel err vs the 2e-2 gate; PSUM
accumulates fp32).  shell*factor is folded host-side and applied during
the PSUM evacuation multiply; outputs return as fp16, unpacked on host.

Charged PE rows: 2m * 8duo * sum_{kh,kw} 8p*win(kh)*win(kw) = 175,232
(~73us at 2.4GHz) vs the previous 16-tile kernel's 12.9M (~5.4ms).
Overlap details: weights stream in 5 pieces per duo with descriptor
generation split across Pool(SWDGE)/SP(HWDGE) so pieces land in chunk
order during pipeline fill; a tiny PE warm-up matmul starts the p-state
ramp during the initial DMA window; the final duo accumulates staged
p-groups (5,2,1) in separate banks so its evacuation overlaps compute.

Sharding: core c takes output row-blocks 2c and 2c+1 (16 of 128 rows);
no inter-core communication.  Timeline-sim: 80,670 ns (baseline
5,468,820 ns).
"""

import sys

for _p in ("/opt/trn_rl_repo",):
    if _p not in sys.path:
        sys.path.insert(0, _p)

import numpy as np

NB = 8
KS = 7
N_CORES = 8
M_PER_CORE = 2
DUOS = 8                   # n-pair duos per m-block
PART = 112                 # (nb2, j8, id7) = (nb2, q8, od7)
SPF = KS * NB * KS         # 392 free: (p, oh, ow)
NWARM = 1                  # PE warm-up matmuls (set pe_busy_start early)
WARMF = 112                # warm-up matmul free size
CH = KS * KS               # 49 (kh, kw) chunks

# chunk order: (3,3) first (full oh window -> start=True covers the
# whole psum tile), rest lexicographic; last chunk carries stop=True.
# Host stores the weight chunks in THIS order so a small prefix DMA
# unblocks the first matmuls.
CHUNKS = [(3, 3)] + [
    (kh, kw) for kh in range(KS) for kw in range(KS) if (kh, kw) != (3, 3)
]
# weight DMA pieces (chunk ranges) and their issuing engines: descriptor
# generation for Pool(SWDGE, ~1us each) and SP(HWDGE, ~0.6us) runs in
# parallel, so interleaving engines keeps the pieces landing in chunk
# order during the pipeline-fill phase
WPIECES = [((0, 6), "pool"), ((6, 16), "sp"), ((16, 27), "sp"),
           ((27, 38), "pool"), ((38, 49), "sp")]
NSLOT = 5                  # k1/weight buffer slots (DMA prefetch depth)
SHF_ENG = "pool"           # engine issuing shell loads
K1_ENG = "sp"              # engine issuing k1 loads
FSPLIT = (5, 2, 1)         # final duo: p-blades per staged group

MODE = "toep16"

_CACHE = {}


def _build_nc(mode):
    import concourse.tile as tile
    from concourse import bacc, mybir

    f16 = mybir.dt.float16
    f32 = mybir.dt.float32

    nc = bacc.Bacc("TRN2", target_bir_lowering=False, debug=False)

    k1r = nc.dram_tensor(
        "k1r", [M_PER_CORE, DUOS, PART, NB * KS * KS], f16,
        kind="ExternalInput"
    )
    # full block-diagonal Toeplitz incl. zeros: walrus requires a 2D
    # weights AP, so the op columns must be contiguous per chunk
    wt = nc.dram_tensor(
        "wt", [M_PER_CORE, DUOS, PART, CH * PART], f16, kind="ExternalInput"
    )
    shf = nc.dram_tensor(
        "shf", [M_PER_CORE, DUOS, PART, SPF], f16, kind="ExternalInput"
    )
    out = nc.dram_tensor(
        "out", [M_PER_CORE, DUOS, PART, SPF], f16, kind="ExternalOutput"
    )

    with tile.TileContext(nc) as tc:
        with (
            tc.tile_pool(name="persist", bufs=1) as persist,
            tc.tile_pool(name="io", bufs=2) as io,
            tc.tile_pool(name="ps", bufs=1, space="PSUM") as pspool,
        ):
            k1t = [
                persist.tile([PART, NB, KS, KS], f16, tag=f"k1t{s}",
                             name=f"k1t{s}")
                for s in range(NSLOT)
            ]
            # weight tile free layout (chunk, col112): lhsT per chunk is
            # the 2D slice [:, c, :]
            wtl = [
                persist.tile([PART, CH, PART], f16, tag=f"wt{s}",
                             name=f"wt{s}")
                for s in range(NSLOT)
            ]
            psum = [
                pspool.tile([128, 512], f32, tag=f"pp{i}", name=f"pp{i}")
                for i in range(4)
            ]


            shslots = [
                persist.tile([PART, SPF], f16, tag=f"shs{i}", name=f"shs{i}")
                for i in range(2)
            ]
            ostslots = [
                persist.tile([PART, SPF], f16, tag=f"osts{i}", name=f"osts{i}")
                for i in range(2)
            ]

            # PE p-state warm-up: the PE runs at ~1/3..1/2 clock for the
            # first 3us of continuous busy.  A dummy matmul on a scratch
            # bank during the initial DMA window starts the ramp clock
            # where the PE would idle anyway.
            warm = persist.tile([PART, max(WARMF, PART)], f16, tag="warm",
                                name="warm")
            nc.vector.memset(warm[:, :], 0.0)
            for _ in range(NWARM):
                nc.tensor.matmul(
                    psum[3][0:PART, 0:WARMF], warm[:, 0:PART],
                    warm[:, 0:WARMF], start=True, stop=True,
                )

            idx = 0
            for m in range(M_PER_CORE):
                for d in range(DUOS):
                    s = idx % NSLOT
                    engs = {"sp": nc.sync, "pool": nc.gpsimd,
                            "act": nc.scalar}
                    engs[K1_ENG].dma_start(
                        out=k1t[s].rearrange("c p h w -> c (p h w)"),
                        in_=k1r[m, d, :, :],
                    )
                    # pipeline-fill duos get their weights in pieces so
                    # early matmuls unblock before the bulk arrives;
                    # steady-state duos prefetch whole tiles (fewer
                    # descriptor gens)
                    pieces = WPIECES
                    for (c0, c1), eng in pieces:
                        issuer = engs[eng]
                        issuer.dma_start(
                            out=wtl[s][:, c0:c1, :].rearrange(
                                "c a b -> c (a b)"
                            ),
                            in_=wt[m, d, :, c0 * PART:c1 * PART],
                        )
                    # shell rides the Pool queue BEHIND the weight pieces:
                    # it is only needed at the duo's end, and issuing it on
                    # Activation would interleave its descriptor gens with
                    # the weight pieces' on the shared HWDGE unit
                    sh = shslots[idx % 2]
                    engs[SHF_ENG].dma_start(out=sh[:, :], in_=shf[m, d, :, :])

                    last = idx == M_PER_CORE * DUOS - 1
                    ost = ostslots[idx % 2]
                    if not last:
                        halves = [(psum[idx % 4], 0, NB)]
                    else:
                        # final duo: accumulate staged p-groups in separate
                        # banks (reusing long-drained rotation banks),
                        # earlier groups' chunks first, so their
                        # evacuation+store overlaps the later groups'
                        # matmuls and only a small store remains at the end
                        halves = []
                        p0 = 0
                        for gi, gsz in enumerate(FSPLIT):
                            halves.append((psum[(idx + gi) % 4], p0, p0 + gsz))
                            p0 += gsz
                    for P, p0, p1 in halves:
                        np_ = p1 - p0
                        fsz = np_ * KS * KS
                        Pv = P[0:PART, 0:fsz].rearrange(
                            "c (p oh ow) -> c p oh ow", p=np_, oh=KS
                        )
                        for ci, (kh, kw) in enumerate(CHUNKS):
                            oh0, oh1 = max(0, 3 - kh), min(KS, 10 - kh)
                            ow0, ow1 = max(0, 3 - kw), min(KS, 10 - kw)
                            dst = Pv[:, :, oh0:oh1, ow0:ow1]
                            lhsT = wtl[s][:, ci, :]
                            rhs = k1t[s][
                                :, p0:p1,
                                oh0 + kh - 3:oh1 + kh - 3,
                                ow0 + kw - 3:ow1 + kw - 3,
                            ]
                            nc.tensor.matmul(
                                dst, lhsT, rhs,
                                start=(ci == 0), stop=(ci == CH - 1),
                            )
                        f0, f1 = p0 * KS * KS, p1 * KS * KS
                        nc.vector.tensor_mul(
                            ost[:, f0:f1], P[0:PART, 0:fsz], sh[:, f0:f1]
                        )
                        # out gets the Activation queue to itself: its
                        # SEQ-stage wait on the evacuation would block
                        # k1/shell prefetch if it shared SP, or weight
                        # prefetch if it shared Pool.  The very last store
                        # rides SP (idle by then, lower DGE latency).
                        out_eng = nc.sync if (last and p1 == NB) else nc.scalar
                        out_eng.dma_start(
                            out=out[m, d, :, f0:f1], in_=ost[:, f0:f1]
                        )
                    idx += 1
    nc.compile()
    return nc


def _get_nc(mode=None):
    if mode is None:
        mode = MODE
    if mode not in _CACHE:
        _CACHE[mode] = _build_nc(mode)
    return _CACHE[mode]


def _prep(k1, k2, shell, factor):
    """Host-side input packing (per-core slices are views of these)."""
    k1 = np.asarray(k1, np.float32).reshape(16, NB, 16, NB, KS, KS, KS)
    k2 = np.asarray(k2, np.float32).reshape(16, NB, 16, NB, KS, KS, KS)
    shell = np.asarray(shell, np.float32).reshape(16, NB, 16, NB, KS, KS, KS)
    f = np.float32(np.asarray(factor).reshape(-1)[0])

    k1h = k1.astype(np.float16)   # [m, p, n, j, d, h, w]
    k2h = k2.astype(np.float16)   # [m, q, n, j, kd, kh, kw]

    # k1r: [m, n, j, id, p, ih, iw] -> (16, 8, 112, 392); no halo padding:
    # the oh/ow windows keep ih/iw interior
    k1r = np.ascontiguousarray(
        k1h.transpose(0, 2, 3, 4, 1, 5, 6)
    ).reshape(16, DUOS, 2 * NB * KS, NB * KS * KS)

    # wt (block-diag Toeplitz): [m, duo, (nb,j,id)=112, chunk=49,
    # (nb',q,od)=112] with the nb==nb' diagonal blocks holding
    # k2[q, j, id-od+3, kh, kw] and zeros elsewhere; the chunk axis is
    # stored in CHUNKS (issue) order
    wt = np.zeros((16, DUOS, 2, NB, KS, CH, 2, NB, KS), np.float16)
    # k2p: [m, duo, nb, j, kd, kh, kw, q]
    k2p = k2h.transpose(0, 2, 3, 4, 5, 6, 1).reshape(
        16, DUOS, 2, NB, KS, KS, KS, NB
    )
    for ci, (kh, kw) in enumerate(CHUNKS):
        for nb in range(2):
            for kd in range(KS):
                for od in range(max(0, 3 - kd), min(KS, 10 - kd)):
                    wt[:, :, nb, :, od + kd - 3, ci, nb, :, od] = \
                        k2p[:, :, nb, :, kd, kh, kw]
    wt = np.ascontiguousarray(wt).reshape(16, DUOS, PART, CH * PART)

    # shf: shell*factor as [m, n, q, od, p, oh, ow] -> (16, 8, 112, 392)
    sh = (shell * f).astype(np.float16).transpose(0, 2, 3, 4, 1, 5, 6)
    sh = np.ascontiguousarray(sh).reshape(16, DUOS, 2 * NB * KS, SPF)

    return k1r, wt, sh


def _make_in_maps(k1, k2, shell, factor):
    k1r, wt, sh = _prep(k1, k2, shell, factor)
    maps = []
    for c in range(N_CORES):
        mlo = c * M_PER_CORE
        maps.append({
            "k1r": np.ascontiguousarray(k1r[mlo:mlo + M_PER_CORE]),
            "wt": np.ascontiguousarray(wt[mlo:mlo + M_PER_CORE]),
            "shf": np.ascontiguousarray(sh[mlo:mlo + M_PER_CORE]),
        })
    return maps


def _gather(results):
    outs = [np.asarray(r["out"]) for r in results]
    full = np.concatenate(outs, axis=0)  # (16, 8, 112, 392) fp16
    full = full.reshape(16, DUOS, 2, NB, KS, NB, KS, KS)
    # [m, duo, nb, q, od, p, oh, ow] -> [m, p, duo, nb, q, od, oh, ow]
    full = full.transpose(0, 5, 1, 2, 3, 4, 6, 7)
    return np.ascontiguousarray(full).reshape(128, 128, KS, KS, KS).astype(
        np.float32
    )


def kernel(k1, k2, shell, factor, _trace=False):
    from concourse.bass_utils import run_bass_kernel_spmd

    nc = _get_nc(MODE)
    in_maps = _make_in_maps(k1, k2, shell, factor)
    try:
        res = run_bass_kernel_spmd(
            nc, in_maps, core_ids=list(range(N_CORES)), trace=_trace
        )
    except ModuleNotFoundError:
        res = run_bass_kernel_spmd(
            nc, in_maps, core_ids=list(range(N_CORES)), trace=False
        )
    out = _gather(res.results)
    if _trace:
        return out, res
    return out
